# revision 1
# baseline (speedup 1.0000x reference)
"""Multi-head attention 3D kernel for Trainium2, 8 NeuronCores.

Problem: x[2, 256, 16, 16, 16] -> MHA(8 heads, head_dim 32) over N=4096
tokens per batch, with QKV projection, softmax attention, out projection,
bias and residual.

Sharding: 8 cores = 2 batches x 4 head-pairs. Each core computes the full
attention for its batch and its 2 heads (Megatron column-split QKV /
row-split out-proj), producing a partial [4096, 256] output. The host sums
the 4 head-pair partials per batch (the "all-reduce"), adds bias and
residual, and reshapes.

Per-core device layout (all fp32):
  - x^T [256, 4096] is DMA'd once; QKV projections run off it.
  - Q^T/K^T are computed replicated across four 32-partition bands
    (bands 0,1 = head0; 2,3 = head1) by host-side column-tiling of the
    weight matrices, so the d=32-contraction score matmuls can use 4x
    row-tiling of the PE array (tile_position=(32r, 0)).
  - Scores are computed transposed, S^T[k, q], softmax'ed without max
    subtraction (scores are O(1) here), exp fused with PSUM eviction on
    the Scalar engine.
  - V is augmented with a ones column so the attention-output matmul also
    produces the softmax denominator (row 32 of O'^T).
  - The out-projection rhs is augmented with a unit column that carries
    the denominator into [q-partition, 1] layout, so the normalization is
    a per-partition tensor_scalar multiply.
"""

import math

import ml_dtypes
import numpy as np

import concourse.bass as bass
import concourse.tile as tile
from concourse import mybir
from concourse.bass_utils import run_bass_kernel_spmd

F32 = mybir.dt.float32
F32R = mybir.dt.float32r
BF16 = mybir.dt.bfloat16
I16 = mybir.dt.int16
SCHR_A = 128.0 / math.log(2.0)  # 2^7/ln2: bf16-bit-domain Schraudolph scale
SCHR_B = 127.0 * 128.0 - 7.0
EMBED = 256
HEADS = 8
HD = 32  # head dim
B = 2
N_TOK = 4096


def build_nc(n_tok: int = N_TOK, reps: int = 1, ablate: str = "") -> bass.Bass:
    """Build the single-core Bass program (same program on all 8 cores).

    reps > 1 wraps the body in a hardware For_i loop (timing harness only).
    """
    assert n_tok % 512 == 0
    n_qc = n_tok // 512  # 512-wide q chunks
    n_kg = n_tok // 256  # k groups of 4 row-tiled 128-tiles (2 per head)
    n_kt = n_tok // 128  # 128-token k tiles

    nc = bass.Bass()
    xT = nc.declare_dram_parameter("xT", [EMBED, n_tok], BF16, isOutput=False)
    wq4 = nc.declare_dram_parameter("wq4", [EMBED, 128], BF16, isOutput=False)
    wk4 = nc.declare_dram_parameter("wk4", [EMBED, 128], BF16, isOutput=False)
    wv2 = nc.declare_dram_parameter("wv2", [EMBED, 64], BF16, isOutput=False)
    wo0 = nc.declare_dram_parameter("wo0", [33, 258], BF16, isOutput=False)
    wo1 = nc.declare_dram_parameter("wo1", [33, 258], BF16, isOutput=False)
    y = nc.declare_dram_parameter("y", [n_tok, EMBED], F32, isOutput=True)

    with tile.TileContext(nc) as tc:
        with (
            tc.tile_pool(name="consts", bufs=1) as consts,
            tc.tile_pool(name="pP", bufs=3) as pP,
            tc.tile_pool(name="pOsb", bufs=2) as pOsb,
            tc.tile_pool(name="pY", bufs=6) as pY,
            tc.tile_pool(name="pR", bufs=4) as pR,
            tc.tile_pool(name="psS", bufs=3, space="PSUM") as psS,
            tc.tile_pool(name="psO", bufs=2, space="PSUM") as psO,
        ):
            import contextlib
            rep_loop = (
                tc.For_i(0, reps, 1, hint_engines=(
                    mybir.EngineType.PE, mybir.EngineType.Activation,
                    mybir.EngineType.DVE, mybir.EngineType.SP,
                ))
                if reps > 1 else contextlib.nullcontext()
            )
            with rep_loop:
                # ---- Phase A: load inputs, project Q^T/K^T (band-replicated) and V ----
                xt = []
                for c in range(2):
                    t = consts.tile([128, n_tok], BF16, tag=f"xt{c}")
                    xt.append(t)
                wq4s, wk4s, wv2s = [], [], []
                for c in range(2):
                    tq = consts.tile([128, 128], BF16, tag=f"wq{c}")
                    nc.sync.dma_start(out=tq, in_=wq4[c * 128 : (c + 1) * 128, :])
                    wq4s.append(tq)
                    tk = consts.tile([128, 128], BF16, tag=f"wk{c}")
                    nc.sync.dma_start(out=tk, in_=wk4[c * 128 : (c + 1) * 128, :])
                    wk4s.append(tk)
                    tv = consts.tile([128, 64], BF16, tag=f"wv{c}")
                    nc.sync.dma_start(out=tv, in_=wv2[c * 128 : (c + 1) * 128, :])
                    wv2s.append(tv)
                wos = []
                for h, wo in enumerate((wo0, wo1)):
                    t = consts.tile([33, 258], BF16, tag=f"wo{h}")
                    nc.sync.dma_start(out=t, in_=wo[:, :])
                    wos.append(t)

                # Q^T/K^T band-replicated: [128, n_tok]; partitions 32r..32r+31
                # hold head (r//2)'s 32-dim Q^T/K^T (r in 0..3).
                # V' per k-tile: [128, 66]; cols 0:32 head0 V, 32 ones,
                # 33:65 head1 V, 65 ones (ones feed the softmax denominator).
                # Interleave Q/K/V chunk production so the attention main loop
                # (and with it the Scalar engine) can start early.
                QT4 = consts.tile([128, n_tok], BF16, tag="QT4")
                KT4 = consts.tile([128, n_tok], BF16, tag="KT4")
                V2p = consts.tile([128, n_kt, 66], BF16, tag="V2p")
                nc.vector.memset(V2p, 1.0)
                def phase_a_chunk(j):
                    """Produce QT4/KT4 512-chunk j and V k-tiles 4j..4j+3."""
                    js = slice(j * 512, (j + 1) * 512)
                    for c in range(2):
                        nc.sync.dma_start(out=xt[c][:, js], in_=xT[c * 128 : (c + 1) * 128, js])
                    for dst, w in ((QT4, wq4s), (KT4, wk4s)):
                        ps = psS.tile([128, 512], F32, tag="ps_big", name="ps_qk")
                        nc.tensor.matmul(
                            ps, lhsT=w[0], rhs=xt[0][:, j * 512 : (j + 1) * 512],
                            start=True, stop=False,
                        )
                        nc.tensor.matmul(
                            ps, lhsT=w[1], rhs=xt[1][:, j * 512 : (j + 1) * 512],
                            start=False, stop=True,
                        )
                        nc.vector.tensor_copy(dst[:, j * 512 : (j + 1) * 512], ps)
                    for kt in range(4 * j, 4 * j + 4):
                        ps = psS.tile([128, 64], F32, tag="ps_big", name="ps_v")
                        nc.tensor.matmul(
                            ps, lhsT=xt[0][:, kt * 128 : (kt + 1) * 128], rhs=wv2s[0],
                            start=True, stop=False,
                        )
                        nc.tensor.matmul(
                            ps, lhsT=xt[1][:, kt * 128 : (kt + 1) * 128], rhs=wv2s[1],
                            start=False, stop=True,
                        )
                        nc.vector.tensor_copy(V2p[:, kt, 0:32], ps[:, 0:32])
                        nc.vector.tensor_copy(V2p[:, kt, 33:65], ps[:, 32:64])

                def use_dve_exp(qc, idx):
                    # DVE takes ~37% of exp tiles (engine balance); fewer during
                    # qc 0, where the Vector engine is busy with projections.
                    if qc == 0:
                        return idx % 8 == 4
                    return idx % 8 in (1, 3, 6)

                def do_kg(qc, kg, pO):
                    qs = slice(qc * 512, (qc + 1) * 512)
                    # Scores S^T for 4 k-tiles (2 per head) via 4x row tiling.
                    # Two half tiles (2 banks each) so ScalarE can stream.
                    pS = [psS.tile([128, 1024], F32, tag="ps_big", name=f"pS{_i}") for _i in range(2)]
                    for r in range(4):
                        if "scores" in ablate:
                            break
                        h, kt = r // 2, 2 * kg + (r % 2)
                        nc.tensor.matmul(
                            pS[r // 2][:, (r % 2) * 512 : (r % 2) * 512 + 512],
                            lhsT=KT4[32 * r : 32 * r + 32, kt * 128 : (kt + 1) * 128],
                            rhs=QT4[32 * r : 32 * r + 32, qs],
                            start=True, stop=True,
                            tile_position=(32 * r, 0),
                        )
                    P4 = pP.tile([128, 2048], BF16, tag="p4")
                    if "exp" in ablate:
                        nc.gpsimd.memset(P4[:, 0:2], 1.0)
                    for half in range(2):
                        if "exp" in ablate:
                            break
                        dst = P4[:, half * 1024 : (half + 1) * 1024]
                        if use_dve_exp(qc, kg * 2 + half):
                            # Schraudolph exp in the bf16 bit domain on the
                            # Vector engine: bitcast(int16(A*x + B)) ~ exp(x).
                            # Element error is a +-4% zero-mean sawtooth that
                            # averages out in the 4096-term attention sums.
                            nc.vector.tensor_scalar(
                                out=dst.bitcast(I16), in0=pS[half],
                                scalar1=SCHR_A, scalar2=SCHR_B,
                                op0=mybir.AluOpType.mult, op1=mybir.AluOpType.add,
                            )
                        else:
                            nc.scalar.activation(
                                dst, pS[half], mybir.ActivationFunctionType.Exp
                            )
                    # 2x column tiling: head0 -> PSUM partitions 0-32,
                    # head1 -> 64-96, concurrent in the 128x64 PE mode.
                    for r in range(4):
                        if "attnv" in ablate:
                            if kg == 0 and r == 0:
                                nc.vector.memset(pO[0:33, :], 1.0)
                                nc.vector.memset(pO[64:97, :], 1.0)
                            break
                        h, kt = r // 2, 2 * kg + (r % 2)
                        nc.tensor.matmul(
                            pO[64 * h : 64 * h + 33, :],
                            lhsT=V2p[:, kt, 33 * h : 33 * h + 33],
                            rhs=P4[:, r * 512 : (r + 1) * 512],
                            start=(kg == 0 and r % 2 == 0),
                            stop=(kg == n_kg - 1 and r % 2 == 1),
                            tile_position=(0, 64 * h),
                        )

                def qc_epilogue(qc, pO):
                    # Evict O'^T (+denominators) to SBUF, out-project, normalize.
                    Osb = [
                        pOsb.tile([33, 512], BF16, tag=f"osb{_h}", name=f"Osb{_h}")
                        for _h in range(2)
                    ]
                    nc.vector.tensor_copy(Osb[0], pO[0:33, :])
                    nc.vector.tensor_copy(Osb[1], pO[64:97, :])
                    for t in range(4):
                        if "epi" in ablate:
                            yd = pY.tile([128, 256], F32, tag="ysb", name="yd")
                            nc.vector.tensor_copy(yd[0:33, :256], Osb[0][:, 0:256])
                            nc.sync.dma_start(out=y[(qc * 4 + t) * 128 : (qc * 4 + t + 1) * 128, :], in_=yd)
                            continue
                        qt = qc * 4 + t
                        ts = slice(t * 128, (t + 1) * 128)
                        yh = []
                        for h in range(2):
                            psy = psO.tile([128, 258], F32, tag="po", name="psy")
                            nc.tensor.matmul(
                                psy,
                                lhsT=Osb[h][:, ts],
                                rhs=wos[h],
                                start=True, stop=True,
                            )
                            rden = pR.tile([128, 1], F32, tag="rden")
                            nc.vector.reciprocal(rden, psy[:, 256:257])
                            ysb = pY.tile([128, 256], F32, tag="ysb")
                            nc.vector.tensor_scalar_mul(ysb, psy[:, 0:256], rden)
                            yh.append(ysb)
                        yout = pY.tile([128, 256], F32, tag="ysb")
                        # y-add on GpSimd: frees Vector-engine cycles for exp.
                        nc.gpsimd.tensor_add(yout, yh[0], yh[1])
                        nc.sync.dma_start(out=y[qt * 128 : (qt + 1) * 128, :], in_=yout)

                # ---- Main loop: attention; phase A interleaved with qc 0.
                # Each qc's epilogue is deferred past the next qc's first two
                # kgs so its PE work doesn't block the exp pipeline at the
                # qc boundary (PE executes in program order).
                pending = None
                for qc in range(n_qc):
                    pO = psO.tile([128, 512], F32, tag="po", name="pO")
                    if qc == 0:
                        for j in range(n_tok // 512):
                            phase_a_chunk(j)
                            for kg in (2 * j, 2 * j + 1):
                                do_kg(0, kg, pO)
                                if kg == 1 and pending is not None:
                                    qc_epilogue(*pending)
                                    pending = None
                    else:
                        for kg in range(n_kg):
                            do_kg(qc, kg, pO)
                            if kg == 1 and pending is not None:
                                qc_epilogue(*pending)
                                pending = None
                    pending = (qc, pO)
                qc_epilogue(*pending)
    _split_multi_waits(nc)
    return nc


def _split_multi_waits(nc, max_waits: int = 1):
    """Walrus in this toolchain accepts at most one sync wait per
    instruction; spill extras onto single-wait NoOps placed just before."""
    for f in nc.m.functions:
        for bb in f.blocks:
            new = []
            for ins in bb.instructions:
                si = ins.sync_info
                if si is not None and si.on_wait and len(si.on_wait) > max_waits:
                    waits = list(si.on_wait)
                    keep, spill = waits[-max_waits:], waits[:-max_waits]
                    for i, w in enumerate(spill):
                        new.append(
                            mybir.InstNoOp(
                                name=f"{ins.name}-w{i}",
                                engine=ins.engine,
                                ins=[], outs=[],
                                debug=ins.debug,
                                sync_info=mybir.SyncInfo(on_wait=[w], on_update=[]),
                            )
                        )
                    ins.sync_info = mybir.SyncInfo(
                        on_wait=keep, on_update=list(si.on_update or [])
                    )
                new.append(ins)
            bb.instructions = new


def kernel(x, W_qkv, W_out, b_out):
    x = np.asarray(x, dtype=np.float32)
    W_out = np.asarray(W_out, dtype=np.float32)
    b_out = np.asarray(b_out, dtype=np.float32)
    nc = build_nc(N_TOK)
    in_maps = make_in_maps(x, W_qkv, W_out, N_TOK)
    res = run_bass_kernel_spmd(nc, in_maps, list(range(8)))
    return gather(res.results, x, b_out)


def make_in_maps(x, W_qkv, W_out, n_tok: int = N_TOK):
    x = np.asarray(x, dtype=np.float32)
    W_qkv = np.asarray(W_qkv, dtype=np.float32)
    W_out = np.asarray(W_out, dtype=np.float32)
    Wq, Wk, Wv = W_qkv[:, 0:EMBED], W_qkv[:, EMBED : 2 * EMBED], W_qkv[:, 2 * EMBED :]
    scale = 1.0 / math.sqrt(HD)
    in_maps = []
    for c in range(8):
        b, hp = c // 4, c % 4
        heads = (2 * hp, 2 * hp + 1)
        wo = []
        for h in heads:
            aug = np.zeros((33, 258), dtype=np.float32)
            aug[:32, :256] = W_out[h * HD : (h + 1) * HD, :]
            aug[32, 256] = 1.0
            wo.append(aug)

        def hcols(W, h):
            return W[:, h * HD : (h + 1) * HD]

        bf = ml_dtypes.bfloat16
        in_maps.append(
            {
                "xT": np.ascontiguousarray(x[b].reshape(EMBED, -1)[:, :n_tok]).astype(bf),
                "wq4": np.ascontiguousarray(
                    np.concatenate(
                        [hcols(Wq, heads[0])] * 2 + [hcols(Wq, heads[1])] * 2, axis=1
                    )
                    * scale
                ).astype(bf),
                "wk4": np.ascontiguousarray(
                    np.concatenate(
                        [hcols(Wk, heads[0])] * 2 + [hcols(Wk, heads[1])] * 2, axis=1
                    )
                ).astype(bf),
                "wv2": np.ascontiguousarray(
                    np.concatenate([hcols(Wv, heads[0]), hcols(Wv, heads[1])], axis=1)
                ).astype(bf),
                "wo0": wo[0].astype(bf),
                "wo1": wo[1].astype(bf),
            }
        )
    return in_maps


def gather(results, x, b_out):
    """Sum head-pair partials per batch, add bias + residual, reshape."""
    x = np.asarray(x, dtype=np.float32)
    b_out = np.asarray(b_out, dtype=np.float32)
    Bb, C, D, H, W = x.shape
    out = np.empty_like(x)
    for b in range(Bb):
        acc = results[4 * b]["y"].astype(np.float32).copy()
        for hp in range(1, 4):
            acc += results[4 * b + hp]["y"]
        acc += b_out[None, :]
        out[b] = x[b] + acc.T.reshape(C, D, H, W)
    return out



# revision 3
# speedup vs baseline: 1.3314x; 1.3314x over previous
"""Multi-head attention 3D kernel for Trainium2, 8 NeuronCores.

Problem: x[2, 256, 16, 16, 16] -> MHA(8 heads, head_dim 32) over N=4096
tokens per batch, with QKV projection, softmax attention, out projection,
bias and residual.

Sharding: 8 cores = 2 batches x 4 head-pairs. Each core computes the full
attention for its batch and its 2 heads (Megatron column-split QKV /
row-split out-proj), producing a partial [4096, 256] output. The host sums
the 4 head-pair partials per batch, adds bias and residual, and reshapes.

Per-core layout: Q^T/K^T band-replicated across four 32-partition bands so
the d=32-contraction score matmuls use 4x row-tiling (concurrent on HW);
V is augmented with a ones column so the attention-output matmul also
produces the softmax denominator; exp is split between ScalarE and a
Schraudolph bf16-bit-domain tensor_scalar on VectorE; the out-projection
rhs carries the denominator so normalization is a per-partition multiply.

v1 emitted, per kg: scores(kg) -> exp(kg) -> attnv(kg). PE executes in
program order, so attnv(kg) (which waits on exp(kg) on Act/DVE) stalled
the PE before scores(kg+1) could issue — the ~1-2us exp latency was paid
on the PE critical path for every kg (~2.2us/kg cadence).

v2 works in half-kg units (one head's [128,1024] score tile) and emits
scores(u+1) BEFORE attnv(u), so the PE always has the next unit's score
matmuls in flight while Act/DVE run exp(u). Steady-state cadence becomes
max(PE ~0.6us, exp-engine ~1.0us) per unit instead of their sum.
"""

import math

import ml_dtypes
import numpy as np

import concourse.bass as bass
import concourse.tile as tile
from concourse import mybir
from concourse.bass_utils import run_bass_kernel_spmd

F32 = mybir.dt.float32
BF16 = mybir.dt.bfloat16
I16 = mybir.dt.int16
SCHR_A = 128.0 / math.log(2.0)  # 2^7/ln2: bf16-bit-domain Schraudolph scale
SCHR_B = 127.0 * 128.0 - 7.0
EMBED = 256
HEADS = 8
HD = 32  # head dim
B = 2
N_TOK = 4096


def build_nc(n_tok: int = N_TOK, reps: int = 1, ablate: str = "", timing_y: bool = False) -> bass.Bass:
    """Build the single-core Bass program (same program on all 8 cores)."""
    assert n_tok % 512 == 0
    n_qc = n_tok // 512  # 512-wide q chunks
    n_kg = n_tok // 256  # k groups: 4 row-tiled 128-tiles (2 per head)
    n_kt = n_tok // 128  # 128-token k tiles
    n_units = n_kg * 2  # per qc: (kg, head) units, one [128,1024] score tile

    nc = bass.Bass()
    xT = nc.declare_dram_parameter("xT", [EMBED, n_tok], BF16, isOutput=False)
    wq4 = nc.declare_dram_parameter("wq4", [EMBED, 128], BF16, isOutput=False)
    wk4 = nc.declare_dram_parameter("wk4", [EMBED, 128], BF16, isOutput=False)
    wv2 = nc.declare_dram_parameter("wv2", [EMBED, 64], BF16, isOutput=False)
    wo0 = nc.declare_dram_parameter("wo0", [33, 258], BF16, isOutput=False)
    wo1 = nc.declare_dram_parameter("wo1", [33, 258], BF16, isOutput=False)
    n_yt = 2 if timing_y else n_tok // 128
    y = nc.declare_dram_parameter("y", [n_yt * 128, EMBED], F32, isOutput=True)

    with tile.TileContext(nc) as tc:
        with (
            tc.tile_pool(name="consts", bufs=1) as consts,
            tc.tile_pool(name="pP", bufs=3) as pP,
            tc.tile_pool(name="pOsb", bufs=2) as pOsb,
            tc.tile_pool(name="pY", bufs=6) as pY,
            tc.tile_pool(name="pR", bufs=4) as pR,
            tc.tile_pool(name="psS", bufs=3, space="PSUM") as psS,
            tc.tile_pool(name="psO", bufs=2, space="PSUM") as psO,
        ):
            import contextlib
            rep_loop = contextlib.nullcontext()
            # reps>1: python-unrolled (For_i hardware loops do not actually
            # iterate under the bass2jax/PJRT path).
            for _rep in range(reps):
             with rep_loop:
                # ---- constants / inputs ----
                xt = [
                    consts.tile([128, n_tok], BF16, tag=f"xt{c}", name=f"xt{c}")
                    for c in range(2)
                ]
                wq4s, wk4s, wv2s = [], [], []
                for c in range(2):
                    tq = consts.tile([128, 128], BF16, tag=f"wq{c}")
                    nc.sync.dma_start(out=tq, in_=wq4[c * 128 : (c + 1) * 128, :])
                    wq4s.append(tq)
                    tk = consts.tile([128, 128], BF16, tag=f"wk{c}")
                    nc.sync.dma_start(out=tk, in_=wk4[c * 128 : (c + 1) * 128, :])
                    wk4s.append(tk)
                    tv = consts.tile([128, 64], BF16, tag=f"wv{c}")
                    nc.sync.dma_start(out=tv, in_=wv2[c * 128 : (c + 1) * 128, :])
                    wv2s.append(tv)
                wos = []
                for h, wo in enumerate((wo0, wo1)):
                    t = consts.tile([33, 258], BF16, tag=f"wo{h}")
                    nc.sync.dma_start(out=t, in_=wo[:, :])
                    wos.append(t)

                QT4 = consts.tile([128, n_tok], BF16, tag="QT4")
                KT4 = consts.tile([128, n_tok], BF16, tag="KT4")
                V2p = consts.tile([128, n_kt, 66], BF16, tag="V2p")
                nc.vector.memset(V2p, 1.0)

                def phase_a_chunk(j):
                    """Produce QT4/KT4 512-chunk j and V k-tiles 4j..4j+3."""
                    js = slice(j * 512, (j + 1) * 512)
                    for c in range(2):
                        nc.sync.dma_start(out=xt[c][:, js], in_=xT[c * 128 : (c + 1) * 128, js])
                    for dst, w in ((QT4, wq4s), (KT4, wk4s)):
                        ps = psS.tile([128, 512], F32, tag="ps_big", name="ps_qk")
                        nc.tensor.matmul(
                            ps, lhsT=w[0], rhs=xt[0][:, js], start=True, stop=False,
                        )
                        nc.tensor.matmul(
                            ps, lhsT=w[1], rhs=xt[1][:, js], start=False, stop=True,
                        )
                        nc.vector.tensor_copy(dst[:, js], ps)
                    for kt in range(4 * j, 4 * j + 4):
                        ps = psS.tile([128, 64], F32, tag="ps_big", name="ps_v")
                        nc.tensor.matmul(
                            ps, lhsT=xt[0][:, kt * 128 : (kt + 1) * 128], rhs=wv2s[0],
                            start=True, stop=False,
                        )
                        nc.tensor.matmul(
                            ps, lhsT=xt[1][:, kt * 128 : (kt + 1) * 128], rhs=wv2s[1],
                            start=False, stop=True,
                        )
                        nc.vector.tensor_copy(V2p[:, kt, 0:32], ps[:, 0:32])
                        nc.vector.tensor_copy(V2p[:, kt, 33:65], ps[:, 32:64])

                def use_dve_exp(qc, idx):
                    # DVE takes 3/8 of exp tiles (engine balance with its
                    # fixed epilogue work); fewer during qc 0, where the
                    # Vector engine is busy with the phase-A projections.
                    if qc == 0:
                        return idx % 8 == 4
                    return idx % 8 in (1, 4, 6)

                # ---- pipelined main loop ----
                # unit u = (qc, kg, h): one [128,1024] score tile = head h's
                # k-tiles (2kg, 2kg+1) x 512 q. Emission order per step:
                #   scores(u+1) [PE] -> exp(u) [Act/DVE] -> attnv(u) [PE]
                units = [
                    (qc, kg, h)
                    for qc in range(n_qc)
                    for kg in range(n_kg)
                    for h in range(2)
                ]
                pS_of = {}      # live score tiles by unit index
                P4_of = {}      # live P4 tiles by (qc, kg)
                pO_of = {}      # psO accumulators by qc
                chunks_done = set()
                pending = None  # deferred epilogue (qc, pO)

                def emit_scores(i):
                    qc, kg, h = units[i]
                    if qc == 0:
                        j = kg // 2
                        if j not in chunks_done:
                            phase_a_chunk(j)
                            chunks_done.add(j)
                    qs = slice(qc * 512, (qc + 1) * 512)
                    pSt = psS.tile([128, 1024], F32, tag="ps_big", name=f"pS{i % 3}")
                    if "scores" not in ablate:
                        for rr in range(2):
                            r = 2 * h + rr
                            kt = 2 * kg + rr
                            nc.tensor.matmul(
                                pSt[:, rr * 512 : rr * 512 + 512],
                                lhsT=KT4[32 * r : 32 * r + 32, kt * 128 : (kt + 1) * 128],
                                rhs=QT4[32 * r : 32 * r + 32, qs],
                                start=True, stop=True,
                                tile_position=(32 * r, 0),
                            )
                    pS_of[i] = pSt

                def emit_exp(i):
                    qc, kg, h = units[i]
                    if h == 0:
                        P4_of[(qc, kg)] = pP.tile(
                            [128, 2048], BF16, tag="p4", name="P4"
                        )
                    P4 = P4_of[(qc, kg)]
                    dst = P4[:, h * 1024 : (h + 1) * 1024]
                    pSt = pS_of.pop(i)
                    if "exp" in ablate:
                        nc.gpsimd.memset(P4[:, h * 1024 : h * 1024 + 2], 1.0)
                        return
                    if use_dve_exp(qc, (kg * 2 + h)):
                        # Schraudolph exp in the bf16 bit domain on the
                        # Vector engine: bitcast(int16(A*x + B)) ~ exp(x).
                        nc.vector.tensor_scalar(
                            out=dst.bitcast(I16), in0=pSt,
                            scalar1=SCHR_A, scalar2=SCHR_B,
                            op0=mybir.AluOpType.mult, op1=mybir.AluOpType.add,
                        )
                    else:
                        nc.scalar.activation(
                            dst, pSt, mybir.ActivationFunctionType.Exp
                        )

                def emit_attnv(i):
                    qc, kg, h = units[i]
                    if qc not in pO_of:
                        pO_of[qc] = psO.tile([128, 512], F32, tag="po", name="pO")
                    pO = pO_of[qc]
                    P4 = P4_of[(qc, kg)]
                    if "attnv" in ablate:
                        if kg == 0 and h == 0:
                            nc.vector.memset(pO[0:33, :], 1.0)
                            nc.vector.memset(pO[64:97, :], 1.0)
                        return
                    for rr in range(2):
                        kt = 2 * kg + rr
                        nc.tensor.matmul(
                            pO[64 * h : 64 * h + 33, :],
                            lhsT=V2p[:, kt, 33 * h : 33 * h + 33],
                            rhs=P4[:, h * 1024 + rr * 512 : h * 1024 + rr * 512 + 512],
                            start=(kg == 0 and rr == 0),
                            stop=(kg == n_kg - 1 and rr == 1),
                            tile_position=(0, 64 * h),
                        )

                def qc_epilogue(qc, pO):
                    # Evict O'^T (+denominators) to SBUF, out-project, normalize.
                    Osb = [
                        pOsb.tile([33, 512], BF16, tag=f"osb{_h}", name=f"Osb{_h}")
                        for _h in range(2)
                    ]
                    nc.vector.tensor_copy(Osb[0], pO[0:33, :])
                    nc.vector.tensor_copy(Osb[1], pO[64:97, :])
                    for t in range(4):
                        if "epi" in ablate:
                            yd = pY.tile([128, 256], F32, tag="ysb", name="yd")
                            nc.vector.tensor_copy(yd[0:33, :256], Osb[0][:, 0:256])
                            qtm = (qc * 4 + t) % n_yt
                            nc.sync.dma_start(out=y[qtm * 128 : (qtm + 1) * 128, :], in_=yd)
                            continue
                        qt = qc * 4 + t
                        ts = slice(t * 128, (t + 1) * 128)
                        yh = []
                        for h in range(2):
                            psy = psO.tile([128, 258], F32, tag="po", name="psy")
                            nc.tensor.matmul(
                                psy, lhsT=Osb[h][:, ts], rhs=wos[h],
                                start=True, stop=True,
                            )
                            rden = pR.tile([128, 1], F32, tag="rden")
                            nc.vector.reciprocal(rden, psy[:, 256:257])
                            ysb = pY.tile([128, 256], F32, tag="ysb")
                            nc.vector.tensor_scalar_mul(ysb, psy[:, 0:256], rden)
                            yh.append(ysb)
                        yout = pY.tile([128, 256], F32, tag="ysb")
                        # y-add on GpSimd: frees Vector-engine cycles for exp.
                        nc.gpsimd.tensor_add(yout, yh[0], yh[1])
                        qtm = qt % n_yt
                        nc.sync.dma_start(out=y[qtm * 128 : (qtm + 1) * 128, :], in_=yout)

                emit_scores(0)
                for i, (qc, kg, h) in enumerate(units):
                    if i + 1 < len(units):
                        emit_scores(i + 1)
                    emit_exp(i)
                    emit_attnv(i)
                    if kg == 1 and h == 1 and pending is not None:
                        qc_epilogue(*pending)
                        pending = None
                    if kg == n_kg - 1 and h == 1:
                        pending = (qc, pO_of.pop(qc))
                        P4_of.clear()
                qc_epilogue(*pending)
    _split_multi_waits(nc)
    return nc


def _split_multi_waits(nc, max_waits: int = 1):
    """Walrus in this toolchain accepts at most one sync wait per
    instruction; spill extras onto single-wait NoOps placed just before."""
    for f in nc.m.functions:
        for bb in f.blocks:
            new = []
            for ins in bb.instructions:
                si = ins.sync_info
                if si is not None and si.on_wait and len(si.on_wait) > max_waits:
                    waits = list(si.on_wait)
                    keep, spill = waits[-max_waits:], waits[:-max_waits]
                    for i, w in enumerate(spill):
                        new.append(
                            mybir.InstNoOp(
                                name=f"{ins.name}-w{i}",
                                engine=ins.engine,
                                ins=[], outs=[],
                                debug=ins.debug,
                                sync_info=mybir.SyncInfo(on_wait=[w], on_update=[]),
                            )
                        )
                    ins.sync_info = mybir.SyncInfo(
                        on_wait=keep, on_update=list(si.on_update or [])
                    )
                new.append(ins)
            bb.instructions = new


def kernel(x, W_qkv, W_out, b_out):
    x = np.asarray(x, dtype=np.float32)
    W_out = np.asarray(W_out, dtype=np.float32)
    b_out = np.asarray(b_out, dtype=np.float32)
    nc = build_nc(N_TOK)
    in_maps = make_in_maps(x, W_qkv, W_out, N_TOK)
    res = run_bass_kernel_spmd(nc, in_maps, list(range(8)))
    return gather(res.results, x, b_out)


def make_in_maps(x, W_qkv, W_out, n_tok: int = N_TOK):
    x = np.asarray(x, dtype=np.float32)
    W_qkv = np.asarray(W_qkv, dtype=np.float32)
    W_out = np.asarray(W_out, dtype=np.float32)
    Wq, Wk, Wv = W_qkv[:, 0:EMBED], W_qkv[:, EMBED : 2 * EMBED], W_qkv[:, 2 * EMBED :]
    scale = 1.0 / math.sqrt(HD)
    in_maps = []
    for c in range(8):
        b, hp = c // 4, c % 4
        heads = (2 * hp, 2 * hp + 1)
        wo = []
        for h in heads:
            aug = np.zeros((33, 258), dtype=np.float32)
            aug[:32, :256] = W_out[h * HD : (h + 1) * HD, :]
            aug[32, 256] = 1.0
            wo.append(aug)

        def hcols(W, h):
            return W[:, h * HD : (h + 1) * HD]

        bf = ml_dtypes.bfloat16
        in_maps.append(
            {
                "xT": np.ascontiguousarray(x[b].reshape(EMBED, -1)[:, :n_tok]).astype(bf),
                "wq4": np.ascontiguousarray(
                    np.concatenate(
                        [hcols(Wq, heads[0])] * 2 + [hcols(Wq, heads[1])] * 2, axis=1
                    )
                    * scale
                ).astype(bf),
                "wk4": np.ascontiguousarray(
                    np.concatenate(
                        [hcols(Wk, heads[0])] * 2 + [hcols(Wk, heads[1])] * 2, axis=1
                    )
                ).astype(bf),
                "wv2": np.ascontiguousarray(
                    np.concatenate([hcols(Wv, heads[0]), hcols(Wv, heads[1])], axis=1)
                ).astype(bf),
                "wo0": wo[0].astype(bf),
                "wo1": wo[1].astype(bf),
            }
        )
    return in_maps


def gather(results, x, b_out):
    """Sum head-pair partials per batch, add bias + residual, reshape."""
    x = np.asarray(x, dtype=np.float32)
    b_out = np.asarray(b_out, dtype=np.float32)
    Bb, C, D, H, W = x.shape
    out = np.empty_like(x)
    for b in range(Bb):
        acc = results[4 * b]["y"].astype(np.float32).copy()
        for hp in range(1, 4):
            acc += results[4 * b + hp]["y"]
        acc += b_out[None, :]
        out[b] = x[b] + acc.T.reshape(C, D, H, W)
    return out
